# revision 6
# baseline (speedup 1.0000x reference)
"""NNUE (HalfKA) forward kernel for Trainium2, data-parallel over batch on 8 cores.

Pipeline per core (1024 samples, tiles of 128):
  1. indirect-DMA gather of 64 embedding rows/sample from a combined
     [22528, 1032] table (ft columns + psqt columns), 8 rows/partition per DMA
  2. DVE tree-reduce -> accumulators [128, 1032] per perspective
  3. stm select, clipped-pairwise activation -> ft [128, 1024]
  4. PE transpose + all-stacks fc0/fc1/fc2 matmuls with host-built one-hot
     bucket masks for the per-sample stack (moe) selection
All scale constants (1/128, 1/64 etc.) are folded into host-prepped weights
where linearity allows.
"""
import numpy as np

import concourse.bacc as bacc
import concourse.bass as bass
import concourse.tile as tile
import concourse.mybir as mybir
from concourse.bass_utils import run_bass_kernel_spmd
from concourse.masks import make_identity

F32 = mybir.dt.float32
I32 = mybir.dt.int32

V = 22528          # ft table rows
FT = 1024          # ft embedding dim
PSQT = 8           # psqt buckets
D = FT + PSQT      # combined gathered row length (1032)
B = 8192
FEATS = 32         # features per bag
NCORES = 8
BC = B // NCORES   # samples per core
P = 128            # partitions
T = BC // P        # sample tiles per core (8)
G = 8              # gathered rows per partition per indirect DMA
NCH = FEATS // G   # chunks per bag per tile (4)
L2 = 15

_CACHE = {}


def _build_nc():
    nc = bacc.Bacc("TRN2", target_bir_lowering=False, debug=False)

    tbl = nc.dram_tensor("tbl", [V, D], F32, kind="ExternalInput")
    widx = nc.dram_tensor("widx", [P, T * FEATS], I32, kind="ExternalInput")
    bidx = nc.dram_tensor("bidx", [P, T * FEATS], I32, kind="ExternalInput")
    m0 = nc.dram_tensor("m0", [BC, 128], F32, kind="ExternalInput")
    m1 = nc.dram_tensor("m1", [BC, 256], F32, kind="ExternalInput")
    m8 = nc.dram_tensor("m8", [BC, 8], F32, kind="ExternalInput")
    stm = nc.dram_tensor("stm", [BC, 1], F32, kind="ExternalInput")
    psqtf = nc.dram_tensor("psqtf", [BC, 1], F32, kind="ExternalInput")
    cbias = nc.dram_tensor("cbias", [1, D], F32, kind="ExternalInput")
    w0T = nc.dram_tensor("w0T", [P, FT], F32, kind="ExternalInput")
    w1T = nc.dram_tensor("w1T", [32, 256], F32, kind="ExternalInput")
    w2T = nc.dram_tensor("w2T", [32, 8], F32, kind="ExternalInput")
    b0 = nc.dram_tensor("b0", [1, 128], F32, kind="ExternalInput")
    b1 = nc.dram_tensor("b1", [1, 256], F32, kind="ExternalInput")
    b2 = nc.dram_tensor("b2", [1, 8], F32, kind="ExternalInput")
    out = nc.dram_tensor("out", [BC, 1], F32, kind="ExternalOutput")

    with tile.TileContext(nc) as tc:
        with tc.tile_pool(name="const", bufs=1) as cp, \
             tc.tile_pool(name="gat", bufs=16) as gpool, \
             tc.tile_pool(name="accs", bufs=2) as apool, \
             tc.tile_pool(name="small", bufs=2) as spool, \
             tc.tile_pool(name="psum", bufs=2, space="PSUM") as pp, \
             tc.tile_pool(name="psacc", bufs=1, space="PSUM") as ppacc:

            # ---- constants, loaded once ----
            ident = cp.tile([P, P], F32)
            make_identity(nc, ident[:])
            widx_sb = cp.tile([P, T * FEATS], I32)
            nc.sync.dma_start(widx_sb[:], widx[:])
            bidx_sb = cp.tile([P, T * FEATS], I32)
            nc.sync.dma_start(bidx_sb[:], bidx[:])
            cbias_sb = cp.tile([P, D], F32)
            nc.sync.dma_start(cbias_sb[:], cbias[:].to_broadcast((P, D)))
            w0T_sb = cp.tile([P, FT], F32)
            nc.sync.dma_start(w0T_sb[:], w0T[:])
            w1T_sb = cp.tile([32, 256], F32)
            nc.sync.dma_start(w1T_sb[:], w1T[:])
            w2T_sb = cp.tile([32, 8], F32)
            nc.sync.dma_start(w2T_sb[:], w2T[:])
            b0_sb = cp.tile([P, 128], F32)
            nc.sync.dma_start(b0_sb[:], b0[:].to_broadcast((P, 128)))
            b1_sb = cp.tile([P, 256], F32)
            nc.sync.dma_start(b1_sb[:], b1[:].to_broadcast((P, 256)))
            b2_sb = cp.tile([P, 8], F32)
            nc.sync.dma_start(b2_sb[:], b2[:].to_broadcast((P, 8)))

            for t in range(T):
                rows = slice(t * P, (t + 1) * P)
                # ---- per-tile small loads ----
                m0_t = spool.tile([P, 128], F32, tag="m0t")
                nc.sync.dma_start(m0_t[:], m0[rows, :])
                m1_t = spool.tile([P, 256], F32, tag="m1t")
                nc.sync.dma_start(m1_t[:], m1[rows, :])
                m8_t = spool.tile([P, 8], F32, tag="m8t")
                nc.sync.dma_start(m8_t[:], m8[rows, :])
                stm_t = spool.tile([P, 1], F32, tag="stmt")
                nc.sync.dma_start(stm_t[:], stm[rows, :])
                psqtf_t = spool.tile([P, 1], F32, tag="psqtft")
                nc.sync.dma_start(psqtf_t[:], psqtf[rows, :])

                # ---- gather + reduce both bags (one row/partition per DMA) ----
                accs = []
                for idx_sb in (widx_sb, bidx_sb):
                    acc = apool.tile([P, D], F32, tag="acc_w" if idx_sb is widx_sb else "acc_b")
                    first = None
                    for s in range(FEATS):
                        buf = gpool.tile([P, D], F32, tag="gather")
                        col = t * FEATS + s
                        nc.gpsimd.indirect_dma_start(
                            out=buf[:],
                            out_offset=None,
                            in_=tbl[:],
                            in_offset=bass.IndirectOffsetOnAxis(
                                ap=idx_sb[:, col:col + 1], axis=0),
                        )
                        if s == 0:
                            first = buf
                        elif s == 1:
                            nc.vector.tensor_add(acc[:], first[:], buf[:])
                        else:
                            nc.vector.tensor_add(acc[:], acc[:], buf[:])
                    accs.append(acc)
                acc_w, acc_b = accs

                # ---- bias, stm select, psqt diff ----
                nc.vector.tensor_add(acc_w[:], acc_w[:], cbias_sb[:])
                nc.vector.tensor_add(acc_b[:], acc_b[:], cbias_sb[:])
                diff = apool.tile([P, D], F32, tag="diff")
                nc.vector.tensor_sub(diff[:], acc_b[:], acc_w[:])
                # psqt partial: sum over bucket-masked diff psqt columns
                pdm = spool.tile([P, 8], F32, tag="pdm")
                nc.vector.tensor_mul(pdm[:], diff[:, FT:D], m8_t[:])
                pd2 = spool.tile([P, 4], F32, tag="pd2")
                nc.vector.tensor_add(pd2[:], pdm[:, 0:4], pdm[:, 4:8])
                nc.vector.tensor_add(pd2[:, 0:2], pd2[:, 0:2], pd2[:, 2:4])
                psel = spool.tile([P, 1], F32, tag="psel")
                nc.vector.tensor_add(psel[:], pd2[:, 0:1], pd2[:, 1:2])
                nc.vector.tensor_mul(psel[:], psel[:], psqtf_t[:])

                # acc_stm = acc_w + stm*(acc_b-acc_w); acc_opp = acc_b - stm*(...)
                nc.vector.tensor_scalar_mul(diff[:, 0:FT], diff[:, 0:FT], stm_t[:, 0:1])
                nc.vector.tensor_add(acc_w[:, 0:FT], acc_w[:, 0:FT], diff[:, 0:FT])
                nc.vector.tensor_sub(acc_b[:, 0:FT], acc_b[:, 0:FT], diff[:, 0:FT])

                # ---- pairwise: ft halves written in place ----
                H = FT // 2
                ft_halves = []
                for acc in (acc_w, acc_b):   # acc_w now holds acc_stm, acc_b holds acc_opp
                    nc.vector.tensor_scalar(
                        out=acc[:, 0:H], in0=acc[:, 0:H],
                        scalar1=0.0, scalar2=127.0,
                        op0=mybir.AluOpType.max, op1=mybir.AluOpType.min)
                    nc.vector.tensor_scalar(
                        out=acc[:, H:FT], in0=acc[:, H:FT],
                        scalar1=0.0, scalar2=127.0,
                        op0=mybir.AluOpType.max, op1=mybir.AluOpType.min)
                    nc.vector.tensor_mul(acc[:, 0:H], acc[:, 0:H], acc[:, H:FT])
                    ft_halves.append(acc[:, 0:H])

                # ---- fc0: transpose ft tiles, matmul all stacks ----
                o0p = ppacc.tile([P, 128], F32, tag="o0p", space="PSUM")
                for k in range(8):
                    half = ft_halves[k // 4]
                    src = half.tensor_slice if False else None
                    col = (k % 4) * P
                    tp = pp.tile([P, P], F32, tag="tpose", space="PSUM")
                    nc.tensor.transpose(tp[:], ft_halves[k // 4][:, col:col + P], ident[:])
                    ftT = spool.tile([P, P], F32, tag="ftT")
                    nc.scalar.copy(ftT[:], tp[:])
                    nc.tensor.matmul(
                        o0p[:], lhsT=ftT[:], rhs=w0T_sb[:, k * P:(k + 1) * P],
                        start=(k == 0), stop=(k == 7))

                # ---- fc0 bias + mask select ----
                o0m = spool.tile([P, 128], F32, tag="o0m")
                nc.vector.tensor_add(o0m[:], o0p[:], b0_sb[:])
                nc.vector.tensor_mul(o0m[:], o0m[:], m0_t[:])
                o0h = spool.tile([P, 64], F32, tag="o0h")
                nc.vector.tensor_add(o0h[:], o0m[:, 0:64], o0m[:, 64:128])
                nc.vector.tensor_add(o0h[:, 0:32], o0h[:, 0:32], o0h[:, 32:64])
                o0 = spool.tile([P, 16], F32, tag="o0")
                nc.vector.tensor_add(o0[:], o0h[:, 0:16], o0h[:, 16:32])

                # ---- slab activations ----
                slab = spool.tile([P, 32], F32, tag="slab")
                nc.vector.memset(slab[:, 30:32], 0.0)
                sq = spool.tile([P, L2], F32, tag="sq")
                nc.vector.tensor_mul(sq[:], o0[:, 0:L2], o0[:, 0:L2])
                nc.vector.tensor_scalar(
                    out=slab[:, 0:L2], in0=sq[:],
                    scalar1=1.0 / 524288.0, scalar2=127.0,
                    op0=mybir.AluOpType.mult, op1=mybir.AluOpType.min)
                nc.vector.tensor_scalar(
                    out=slab[:, L2:2 * L2], in0=o0[:, 0:L2],
                    scalar1=1.0 / 64.0, scalar2=0.0,
                    op0=mybir.AluOpType.mult, op1=mybir.AluOpType.max)
                nc.vector.tensor_scalar_min(slab[:, L2:2 * L2], slab[:, L2:2 * L2], 127.0)

                # ---- fc1 ----
                tps = pp.tile([32, P], F32, tag="tpose32", space="PSUM")
                nc.tensor.transpose(tps[:], slab[:], ident[:])
                slabT = spool.tile([32, P], F32, tag="slabT")
                nc.scalar.copy(slabT[:], tps[:])
                o1p = ppacc.tile([P, 256], F32, tag="o1p", space="PSUM")
                nc.tensor.matmul(o1p[:], lhsT=slabT[:], rhs=w1T_sb[:], start=True, stop=True)
                o1m = spool.tile([P, 256], F32, tag="o1m")
                nc.vector.tensor_add(o1m[:], o1p[:], b1_sb[:])
                nc.vector.tensor_mul(o1m[:], o1m[:], m1_t[:])
                o1h = spool.tile([P, 128], F32, tag="o1h")
                nc.vector.tensor_add(o1h[:], o1m[:, 0:128], o1m[:, 128:256])
                nc.vector.tensor_add(o1h[:, 0:64], o1h[:, 0:64], o1h[:, 64:128])
                o1 = spool.tile([P, 32], F32, tag="o1")
                nc.vector.tensor_add(o1[:], o1h[:, 0:32], o1h[:, 32:64])
                nc.vector.tensor_scalar(
                    out=o1[:], in0=o1[:],
                    scalar1=1.0 / 64.0, scalar2=0.0,
                    op0=mybir.AluOpType.mult, op1=mybir.AluOpType.max)
                nc.vector.tensor_scalar_min(o1[:], o1[:], 127.0)

                # ---- fc2 ----
                tpa = pp.tile([32, P], F32, tag="tpose32", space="PSUM")
                nc.tensor.transpose(tpa[:], o1[:], ident[:])
                ac1T = spool.tile([32, P], F32, tag="ac1T")
                nc.scalar.copy(ac1T[:], tpa[:])
                o2p = ppacc.tile([P, 8], F32, tag="o2p", space="PSUM")
                nc.tensor.matmul(o2p[:], lhsT=ac1T[:], rhs=w2T_sb[:], start=True, stop=True)
                o2m = spool.tile([P, 8], F32, tag="o2m")
                nc.vector.tensor_add(o2m[:], o2p[:], b2_sb[:])
                nc.vector.tensor_mul(o2m[:], o2m[:], m8_t[:])
                o2h = spool.tile([P, 4], F32, tag="o2h")
                nc.vector.tensor_add(o2h[:], o2m[:, 0:4], o2m[:, 4:8])
                nc.vector.tensor_add(o2h[:, 0:2], o2h[:, 0:2], o2h[:, 2:4])
                res = spool.tile([P, 1], F32, tag="res")
                nc.vector.tensor_add(res[:], o2h[:, 0:1], o2h[:, 1:2])

                # ---- skip + psqt + output ----
                skip = spool.tile([P, 1], F32, tag="skip")
                nc.vector.tensor_scalar_mul(skip[:], o0[:, L2:16], 9600.0 / 8128.0 / 16.0)
                nc.vector.tensor_add(res[:], res[:], skip[:])
                nc.vector.tensor_add(res[:], res[:], psel[:])
                nc.sync.dma_start(out[rows, :], res[:])

    nc.compile()
    return nc


def _prep_inputs(inputs):
    """Host-side prep: combined table, transposed/prescaled weights, per-core
    index layouts and bucket one-hot masks."""
    ft_w = np.ascontiguousarray(inputs["ft_w"], dtype=np.float32)
    psqt_w = np.ascontiguousarray(inputs["psqt_w"], dtype=np.float32)
    tbl = np.concatenate([ft_w, psqt_w], axis=1)          # [V, 1032]
    tbl = np.ascontiguousarray(tbl)

    ft_bias = np.asarray(inputs["ft_bias"], dtype=np.float32)
    cbias = np.concatenate([ft_bias, np.zeros(PSQT, np.float32)]).reshape(1, D)

    fc0_w = np.asarray(inputs["fc0_w"], dtype=np.float32)  # [8,16,1024]
    fc1_w = np.asarray(inputs["fc1_w"], dtype=np.float32)  # [8,32,32]
    fc2_w = np.asarray(inputs["fc2_w"], dtype=np.float32)  # [8,1,32]
    # w0T in SBUF layout [128 h-part, 8 tiles * 128 (s,j)], scaled by 1/128
    a = fc0_w.transpose(2, 0, 1).reshape(FT, 128) * (1.0 / 128.0)   # [h, (s,j)]
    w0T = np.ascontiguousarray(
        a.reshape(8, 128, 128).transpose(1, 0, 2).reshape(128, FT))
    w1T = np.ascontiguousarray(fc1_w.transpose(2, 0, 1).reshape(32, 256))
    w2T = np.ascontiguousarray(fc2_w[:, 0, :].T * (1.0 / 16.0))     # [32, 8]
    b0 = np.asarray(inputs["fc0_b"], np.float32).reshape(1, 128)
    b1 = np.asarray(inputs["fc1_b"], np.float32).reshape(1, 256)
    b2 = np.asarray(inputs["fc2_b"], np.float32).reshape(1, 8) * (1.0 / 16.0)

    w_feats = np.asarray(inputs["w_feats"]).astype(np.int32)
    b_feats = np.asarray(inputs["b_feats"]).astype(np.int32)
    stm = np.asarray(inputs["stm"]).astype(np.float32)
    bucket = np.asarray(inputs["bucket"]).astype(np.int64)

    in_maps = []
    for c in range(NCORES):
        s = slice(c * BC, (c + 1) * BC)
        wf = w_feats[c * BC * FEATS:(c + 1) * BC * FEATS].reshape(T, P, FEATS)
        bf = b_feats[c * BC * FEATS:(c + 1) * BC * FEATS].reshape(T, P, FEATS)
        widx = np.ascontiguousarray(wf.transpose(1, 0, 2).reshape(P, T * FEATS))
        bidx = np.ascontiguousarray(bf.transpose(1, 0, 2).reshape(P, T * FEATS))
        bk = bucket[s]
        m0 = (bk[:, None] == (np.arange(128) // 16)).astype(np.float32)
        m1 = (bk[:, None] == (np.arange(256) // 32)).astype(np.float32)
        m8 = (bk[:, None] == np.arange(8)).astype(np.float32)
        st = stm[s].reshape(BC, 1)
        psqtf = (st - 0.5) * (1.0 / 16.0)
        in_maps.append({
            "tbl": tbl, "widx": widx, "bidx": bidx,
            "m0": m0, "m1": m1, "m8": m8,
            "stm": st, "psqtf": psqtf.astype(np.float32),
            "cbias": cbias, "w0T": w0T, "w1T": w1T, "w2T": w2T,
            "b0": b0, "b1": b1, "b2": b2,
        })
    return in_maps


def kernel(**inputs) -> np.ndarray:
    if "nc" not in _CACHE:
        _CACHE["nc"] = _build_nc()
    nc = _CACHE["nc"]
    in_maps = _prep_inputs(inputs)
    r = run_bass_kernel_spmd(nc, in_maps, core_ids=list(range(NCORES)))
    return np.concatenate([r.results[c]["out"][:, 0] for c in range(NCORES)])
